# revision 39
# baseline (speedup 1.0000x reference)
"""Trainium2 Bass kernel for AttnBlock3D (GroupNorm + single-head attention + residual).

Sharding: 8 cores; core i handles batch i//4, query-token slice i%4 (1024 of
N=4096 tokens). Each core receives its batch's full (C=256, N=4096) x slab,
*rolled* so its query tokens come first (keeps the SPMD program identical
across cores), computes GroupNorm + full V locally, attention for its query
slice only, and writes a (256, 1024) output slice. The host reassembles the
slices. No collectives.

Device algorithm (HW-measured ~70 us/execution on a TRN2 NeuronCore):
  - GroupNorm via per-channel bn_stats overlapped with the chunked x DMA
    (both HWDGE rings); the 8-channel group merge runs through a tiny
    selector matmul (groups never cross the two 128-partition blocks).
  - h is fp8e4 in an mb-major layout [128, 4, 2, 128] per 512-column chunk:
    it serves directly as the stationary operand of the scores matmul and,
    sliced, as the plain-matmul operand for the q/v projections.
  - No k tensor exists: scores are reassociated as sT = h^T (wk^T q).
    r = wk^T q is a tiny per-query-pass transform (q is only 256x1024);
    the bk bias contributes a per-column constant that the softmax cancels,
    so it is exactly dropped.
  - scores/AV/denominator matmuls run fp8e4 DoubleRow (2 MACs/cell/cycle),
    f32 PSUM accumulation. Softmax needs no max-subtraction (|scores| < ~7);
    exp outputs are scaled by 1/4 (bias -ln4) to stay under the TRN fp8e4
    max of 240 - the uniform scale cancels in the normalization, which is
    folded into the final output pass (projection is linear per column).
  - One exp ACT instruction per 4 key-blocks (1024 columns) amortizes the
    ~352-cycle ACT overhead; once attention starts, ACT runs Exp only
    (any other func would force a ~1.3us activation-table reload).
  - V production (plain fp8 matmuls from h) is fused into the first query
    pass, prefetched two chunks ahead, with PSUM rounds alternating between
    two pools so they never serialize behind the exp-held score slots.
"""

import os
import sys

import numpy as np

for _p in ("/opt/trn_rl_repo", "/opt/pypackages"):
    if os.path.isdir(_p) and _p not in sys.path:
        sys.path.append(_p)

import contextlib
import ml_dtypes
from contextlib import ExitStack

import concourse.bass as bass
import concourse.bacc as bacc
import concourse.tile as tile
from concourse import mybir
from concourse.bass_utils import run_bass_kernel_spmd

F32 = mybir.dt.float32
BF16 = mybir.dt.bfloat16
FP8 = mybir.dt.float8e4
DR = mybir.MatmulPerfMode.DoubleRow
EXP_BIAS = -1.3862944  # -ln(4)
NPBF16 = ml_dtypes.bfloat16
NPF8 = ml_dtypes.float8_e4m3fn
AF = mybir.ActivationFunctionType
ALU = mybir.AluOpType

C = 256          # channels
N = 4096         # tokens per batch (16*16*16)
NQ = 1024        # query tokens per core
NCB = 2          # channel blocks of 128
GPB = 16         # groups per channel block (32 groups of 8 channels total)
GS = 8           # channels per group
CHUNK = 512      # x / h / k production chunk (columns)
QC = 256         # query-column chunk per attention pass
NQC = NQ // QC
MB = N // 128    # key/value token blocks
G = 4            # key-blocks per exp instruction (one production chunk)

ATTN_SCALE = C ** -0.5
VEC = {"gnw": 0, "gnb": 1, "bq": 2, "bk": 3, "bp": 4}

TRACE = False
LAST_RESULTS = None
KABL = int(os.environ.get("KABL", "0"))  # 1=GN only, 2=+qkv, 0=full
KQKV = os.environ.get("KQKV", "dve")  # dve | act | split: engine for q PSUM->SBUF copies


def _emit(nc: bass.Bass, reps: int = 1):
    xb_d = nc.dram_tensor("xb", [NCB, 128, N], BF16, kind="ExternalInput").ap()
    # fp8 blob: [128, wq8(2,2,128) | wk8(2,2,128) | wv8(2,256)] = [128, 3, 2, 256]
    w8_d = nc.dram_tensor("w8", [128, 3, NCB, C], FP8, kind="ExternalInput").ap()
    wpt_d = nc.dram_tensor("wpt", [128, NCB, C], BF16, kind="ExternalInput").ap()
    vecs_d = nc.dram_tensor("vecs", [128, NCB, len(VEC)], F32, kind="ExternalInput").ap()
    bv_d = nc.dram_tensor("bv", [1, 2 * C], F32, kind="ExternalInput").ap()
    sel_d = nc.dram_tensor("sel", [128, GPB], F32, kind="ExternalInput").ap()  # entries 1/GS
    selT_d = nc.dram_tensor("selT", [GPB, 128], F32, kind="ExternalInput").ap()
    out_d = nc.dram_tensor("out", [NCB, 128, NQ], F32, kind="ExternalOutput").ap()

    with tile.TileContext(nc) as tc, ExitStack() as ctx:
      persist = ctx.enter_context(tc.tile_pool(name="persist", bufs=1))
      work = ctx.enter_context(tc.tile_pool(name="work", bufs=3))
      gnp = ctx.enter_context(tc.tile_pool(name="gnp", bufs=2))
      psA = ctx.enter_context(tc.tile_pool(name="psA", bufs=1, space="PSUM"))
      psS = ctx.enter_context(tc.tile_pool(name="psS", bufs=2, space="PSUM"))
      psAV = ctx.enter_context(tc.tile_pool(name="psAV", bufs=1, space="PSUM"))
      loop_cm = tc.For_i(0, reps, 1) if reps > 1 else contextlib.nullcontext()
      with loop_cm:
       for _rep in range(1):
        # ---- x DMA in 1024-col chunks alternating both HWDGE rings;
        # bn_stats (512-wide) overlaps the transfer
        NXC = N // CHUNK  # 8 x-chunks of 512 per channel block
        XD = 512
        x_sb = [[None] * (N // XD) for _ in range(NCB)]
        stats = []
        for cb in range(NCB):
            st = gnp.tile([128, NXC, 6], F32, tag=f"bnstats{cb}", name=f"bnstats{cb}")
            stats.append(st)
            for i in range(N // XD):
                t = persist.tile([128, XD], BF16, tag=f"x{cb}_{i}", name=f"x{cb}_{i}")
                ring = nc.sync if (cb * (N // XD) + i) % 2 == 0 else nc.scalar
                ring.dma_start(out=t, in_=xb_d[cb][:, i * XD : (i + 1) * XD])
                x_sb[cb][i] = t
                for hh in range(XD // CHUNK):
                    nc.vector.bn_stats(
                        out=st[:, i * (XD // CHUNK) + hh, :],
                        in_=t[:, hh * CHUNK : (hh + 1) * CHUNK],
                    )

        XPC = XD // CHUNK

        def xchunk(cb, i):
            # (128, CHUNK) view of the i-th 512-col chunk
            return x_sb[cb][i // XPC][:, (i % XPC) * CHUNK : (i % XPC + 1) * CHUNK]

        if KABL == 4:
            for ob in range(NCB):
                o = work.tile([128, CHUNK], F32, tag="o_abl4", name="o_abl4")
                nc.vector.tensor_copy(out=o, in_=xchunk(ob, 0))
                nc.sync.dma_start(out=out_d[ob][:, 0:CHUNK], in_=o)
            continue

        # ---- parameter loads
        w8_blob = persist.tile([128, 3, NCB, C], FP8, tag="w8", name="w8")
        nc.sync.dma_start(out=w8_blob, in_=w8_d)
        w8_sb = {
            "wq8": w8_blob[:, 0].rearrange("p a (b c) -> p a b c", b=NCB),
            "wk8": w8_blob[:, 1].rearrange("p a (b c) -> p a b c", b=NCB),
        }
        wv8_sb = w8_blob[:, 2]
        wpt_t = persist.tile([128, NCB, C], BF16, tag="wpt", name="wpt")
        nc.scalar.dma_start(out=wpt_t, in_=wpt_d)
        wpt_sb = [wpt_t[:, cb, :] for cb in range(NCB)]
        vecs_t = persist.tile([128, NCB, len(VEC)], F32, tag="vecs", name="vecs")
        nc.sync.dma_start(out=vecs_t, in_=vecs_d)
        vecs_sb = [vecs_t[:, cb, :] for cb in range(NCB)]

        def vec(cb, name):
            return vecs_sb[cb][:, VEC[name] : VEC[name] + 1]

        bvb22 = persist.tile([128, NCB, 2, 128], F32, tag="bvb22")
        nc.gpsimd.dma_start(
            out=bvb22,
            in_=bass.AP(tensor=bv_d.tensor, offset=bv_d.offset, ap=[[0, 128], [1, 2 * C]]),
        )
        sel_sb = persist.tile([128, GPB], F32, tag="sel")
        nc.scalar.dma_start(out=sel_sb, in_=sel_d)
        selT_sb = persist.tile([GPB, 128], F32, tag="selT")
        nc.scalar.dma_start(out=selT_sb, in_=selT_d)
        ones_pad = persist.tile([128, 2, 16], FP8, tag="ones_pad")
        nc.vector.memset(ones_pad, 1.0)
        ones_col = ones_pad[:, :, 0:1]
        ones_row = persist.tile([1, 128], BF16, tag="ones_row")
        nc.vector.memset(ones_row, 1.0)
        eps16 = persist.tile([GPB, 1], F32, tag="eps16")
        nc.vector.memset(eps16, 1e-5)
        expb = persist.tile([128, 1], F32, tag="expb")
        nc.vector.memset(expb, EXP_BIAS)

        # ---- GroupNorm scales/biases, both channel blocks in shared ops
        mv = gnp.tile([128, NCB, 2], F32, tag="mv")
        for cb in range(NCB):
            nc.vector.bn_aggr(out=mv[:, cb, :], in_=stats[cb])
        rhs6 = gnp.tile([128, NCB, 3], F32, tag="rhs6")
        nc.vector.tensor_copy(out=rhs6[:, :, 0:2], in_=mv)
        nc.vector.tensor_mul(
            rhs6[:, :, 2:3].rearrange("p a b -> p (a b)"),
            mv[:, :, 0:1].rearrange("p a b -> p (a b)"),
            mv[:, :, 0:1].rearrange("p a b -> p (a b)"),
        )
        # sel entries are 1/GS -> group averages of [mean, var, mean^2]
        gsum_ps = psA.tile([GPB, NCB, 3], F32, tag="mm512")
        nc.tensor.matmul(
            out=gsum_ps.rearrange("p a b -> p (a b)"),
            lhsT=sel_sb,
            rhs=rhs6.rearrange("p a b -> p (a b)"),
            start=True,
            stop=True,
        )
        gsum = gnp.tile([GPB, NCB, 3], F32, tag="gsum")
        nc.vector.tensor_copy(out=gsum, in_=gsum_ps)
        gv = gnp.tile([GPB, NCB, 1], F32, tag="gv")
        nc.vector.tensor_add(
            gv.rearrange("p a b -> p (a b)"),
            gsum[:, :, 1].rearrange("p a -> p a"),
            gsum[:, :, 2].rearrange("p a -> p a"),
        )
        gm2 = gnp.tile([GPB, NCB, 1], F32, tag="gm2")
        nc.vector.tensor_mul(
            gm2.rearrange("p a b -> p (a b)"),
            gsum[:, :, 0].rearrange("p a -> p a"),
            gsum[:, :, 0].rearrange("p a -> p a"),
        )
        nc.vector.tensor_sub(
            gv.rearrange("p a b -> p (a b)"),
            gv.rearrange("p a b -> p (a b)"),
            gm2.rearrange("p a b -> p (a b)"),
        )
        gs2 = gnp.tile([GPB, NCB, 2], F32, tag="gs2")
        nc.scalar.activation(
            out=gs2[:, :, 1].rearrange("p a -> p a"),
            in_=gv.rearrange("p a b -> p (a b)"),
            func=AF.Sqrt,
            bias=eps16,
            scale=1.0,
        )
        nc.vector.reciprocal(
            gs2[:, :, 1].rearrange("p a -> p a"),
            gs2[:, :, 1].rearrange("p a -> p a"),
        )
        nc.vector.tensor_copy(
            out=gs2[:, :, 0].rearrange("p a -> p a"),
            in_=gsum[:, :, 0].rearrange("p a -> p a"),
        )
        cst = psA.tile([128, NCB, 2], F32, tag="mm512")
        nc.tensor.matmul(
            out=cst.rearrange("p a b -> p (a b)"),
            lhsT=selT_sb,
            rhs=gs2.rearrange("p a b -> p (a b)"),
            start=True,
            stop=True,
        )
        gnwv = gnp.tile([128, NCB, 1], F32, tag="gnwv")
        nc.vector.tensor_copy(out=gnwv[:, 0, :], in_=vec(0, "gnw"))
        nc.vector.tensor_copy(out=gnwv[:, 1, :], in_=vec(1, "gnw"))
        scv = persist.tile([128, NCB, 1], F32, tag="scv")
        nc.vector.tensor_mul(
            scv.rearrange("p a b -> p (a b)"),
            cst[:, :, 1].rearrange("p a -> p a"),
            gnwv.rearrange("p a b -> p (a b)"),
        )
        tmp = gnp.tile([128, NCB, 1], F32, tag="tmpb")
        nc.vector.tensor_mul(
            tmp.rearrange("p a b -> p (a b)"),
            cst[:, :, 0].rearrange("p a -> p a"),
            scv.rearrange("p a b -> p (a b)"),
        )
        gnbv = gnp.tile([128, NCB, 1], F32, tag="gnbv")
        nc.vector.tensor_copy(out=gnbv[:, 0, :], in_=vec(0, "gnb"))
        nc.vector.tensor_copy(out=gnbv[:, 1, :], in_=vec(1, "gnb"))
        nbv = persist.tile([128, NCB, 1], F32, tag="nbv")
        nc.vector.tensor_sub(
            nbv.rearrange("p a b -> p (a b)"),
            gnbv.rearrange("p a b -> p (a b)"),
            tmp.rearrange("p a b -> p (a b)"),
        )
        scale_c = [scv[:, cb, :] for cb in range(NCB)]
        nbias_c = [nbv[:, cb, :] for cb in range(NCB)]

        # ---- h (fp8, packed [128, 2, CHUNK] per chunk tile) streamed per chunk
        h8 = []
        for i in range(NXC):
            t = persist.tile([128, NCB, CHUNK], FP8, tag=f"h8_{i}", name=f"h8_{i}")
            for cb in range(NCB):
                if (i * NCB + cb) % 2 == 0:
                    nc.vector.tensor_scalar(
                        out=t[:, cb, :],
                        in0=xchunk(cb, i),
                        scalar1=scale_c[cb],
                        scalar2=nbias_c[cb],
                        op0=ALU.mult,
                        op1=ALU.add,
                    )
                else:
                    nc.scalar.activation(
                        out=t[:, cb, :],
                        in_=xchunk(cb, i),
                        func=AF.Identity,
                        scale=scale_c[cb],
                        bias=nbias_c[cb],
                    )
            h8.append(t)

        if KABL == 1:
            for ob in range(NCB):
                o = work.tile([128, CHUNK], F32, tag="o_abl", name="o_abl")
                nc.vector.tensor_copy(out=o, in_=h8[0][:, ob, :])
                nc.sync.dma_start(out=out_d[ob][:, 0:CHUNK], in_=o)
            continue

        # ---- projections (all fp8 DoubleRow): k, vT first (attention needs
        # them in full), then q
        # per-512-key-chunk tiles so production streams into the attention loop
        k8 = [
            persist.tile([128, 4, NCB, 128], FP8, tag=f"k8_{c}", name=f"k8_{c}")
            for c in range(NXC)
        ]
        q8 = persist.tile([128, NQC, NCB, QC], FP8, tag="q8")
        vT8 = [
            persist.tile([128, 2, NCB, 2, 128], FP8, tag=f"vT8_{c}", name=f"vT8_{c}")
            for c in range(NXC)
        ]

        def copy_bias(idx, out, ps, bias_ap):
            use_act = KQKV == "act" or (KQKV == "split" and idx % 2 == 0)
            if use_act:
                nc.scalar.activation(out=out, in_=ps, func=AF.Identity, bias=bias_ap, scale=1.0)
            else:
                nc.vector.tensor_scalar_add(out=out, in0=ps, scalar1=bias_ap)

        def produce_kv(mch):
            for ob in range(NCB):
                ps = psS.tile([128, CHUNK], F32, tag="s4", name="k_ps")
                nc.tensor.matmul(
                    out=ps,
                    lhsT=w8_sb["wk8"][:, ob],
                    rhs=h8[mch],
                    perf_mode=DR,
                    start=True,
                    stop=True,
                )
                copy_bias(mch * NCB + ob,
                          k8[mch][:, :, ob, :],
                          ps.rearrange("p (a b) -> p a b", a=4), vec(ob, "bk"))
            for jl in range(2):
                ps = psS.tile([128, 2, C], F32, tag="s4", name="vt_ps")
                for mmt in range(2):
                    mm = 2 * jl + mmt
                    for ci in range(NCB):
                        nc.tensor.matmul(
                            out=ps[:, mmt, :],
                            lhsT=h8[mch][:, ci, mm * 128 : (mm + 1) * 128],
                            rhs=wv8_sb[:, ci, :],
                            start=(ci == 0),
                            stop=(ci == 1),
                        )
                nc.vector.tensor_add(
                    vT8[mch][:, jl],
                    ps.rearrange("p t (cb cc) -> p cb t cc", cb=NCB),
                    bvb22,
                )

        for hc in range(NQ // CHUNK):
            for ob in range(NCB):
                ps = psA.tile([128, CHUNK], F32, tag="mm512")
                nc.tensor.matmul(
                    out=ps,
                    lhsT=w8_sb["wq8"][:, ob],
                    rhs=h8[hc],
                    perf_mode=DR,
                    start=True,
                    stop=True,
                )
                copy_bias(hc * NCB + ob + 1,
                          q8[:, 2 * hc : 2 * hc + 2, ob, :],
                          ps.rearrange("p (a b) -> p a b", a=2), vec(ob, "bq"))

        # ---- residual (+ output-proj bias), staged per channel block
        xres = []
        for ob in range(NCB):
            t = persist.tile([128, NQ], F32, tag=f"xres{ob}", name=f"xres{ob}")
            for i in range(NQ // CHUNK):
                nc.gpsimd.tensor_scalar_add(
                    out=t[:, i * CHUNK : (i + 1) * CHUNK],
                    in0=xchunk(ob, i),
                    scalar1=vec(ob, "bp"),
                )
            xres.append(t)

        if KABL == 2:
            for ob in range(NCB):
                nc.sync.dma_start(out=out_d[ob], in_=xres[ob])
            continue

        # ---- attention: 4 query passes of 256 columns; one exp instruction
        # per 4 key-blocks; k/vT production prefetched 2 chunks ahead during
        # the first pass
        for qc in range(NQC):
            qs = q8[:, qc]
            av_ps = [
                psAV.tile([128, QC], F32, tag=f"av{cb}", name=f"av{cb}")
                for cb in range(NCB)
            ]
            sum_acc = psA.tile([1, QC], F32, tag="sum_acc", name="sum_acc", bufs=1)
            NG = MB // G

            def emit_scores(g):
                if qc == 0 and g + 3 < NXC:
                    produce_kv(g + 3)
                s4 = psS.tile([128, G, QC], F32, tag="s4", name="s4")
                for t in range(G):
                    mb = G * g + t
                    nc.tensor.matmul(
                        out=s4[:, t, :],
                        lhsT=k8[mb // 4][:, mb % 4],
                        rhs=qs,
                        perf_mode=DR,
                        start=True,
                        stop=True,
                    )
                return s4

            if qc == 0:
                for _c in range(3):
                    produce_kv(_c)
            s4 = emit_scores(0)
            for g in range(NG):
                eT = work.tile([128, G, QC], FP8, tag="eT")
                nc.scalar.activation(
                    out=eT, in_=s4, func=AF.Exp, scale=ATTN_SCALE, bias=expb
                )
                if g + 1 < NG:
                    s4 = emit_scores(g + 1)
                for p in range(G // 2):
                    for cb in range(NCB):
                        nc.tensor.matmul(
                            out=av_ps[cb],
                            lhsT=vT8[g][:, p, cb],
                            rhs=eT[:, 2 * p : 2 * p + 2, :],
                            perf_mode=DR,
                            start=(g == 0 and p == 0),
                            stop=(g == NG - 1 and p == G // 2 - 1),
                        )
                for p in range(G // 2):
                    nc.tensor.matmul(
                        out=sum_acc,
                        lhsT=ones_col,
                        rhs=eT[:, 2 * p : 2 * p + 2, :],
                        perf_mode=DR,
                        start=(g == 0 and p == 0),
                        stop=(g == NG - 1 and p == G // 2 - 1),
                    )
            inv = work.tile([1, QC], F32, tag="inv")
            nc.vector.reciprocal(inv, sum_acc)
            inv_b = work.tile([1, QC], BF16, tag="inv_b")
            nc.vector.tensor_copy(out=inv_b, in_=inv)
            invb_ps = psA.tile([128, QC], F32, tag="mm512", name="invb_ps")
            nc.tensor.matmul(out=invb_ps, lhsT=ones_row, rhs=inv_b, start=True, stop=True)
            invb = work.tile([128, QC], F32, tag="invb_sb")
            nc.vector.tensor_copy(out=invb, in_=invb_ps)
            av_sb = work.tile([128, NCB, QC], BF16, tag="avsb", name="avsb")
            for cb in range(NCB):
                nc.vector.tensor_copy(out=av_sb[:, cb, :], in_=av_ps[cb])
            for ob in range(NCB):
                pj = psA.tile([128, QC], F32, tag="mm512", name="pj")
                for cb in range(NCB):
                    nc.tensor.matmul(
                        out=pj,
                        lhsT=wpt_sb[cb][:, ob * 128 : (ob + 1) * 128],
                        rhs=av_sb[:, cb, :],
                        start=(cb == 0),
                        stop=(cb == 1),
                    )
                t1 = work.tile([128, QC], F32, tag="t1")
                nc.vector.tensor_mul(t1, pj, invb)
                o = work.tile([128, QC], F32, tag="o")
                nc.vector.tensor_add(o, t1, xres[ob][:, qc * QC : (qc + 1) * QC])
                ring = nc.sync if ob == 0 else nc.scalar
                ring.dma_start(out=out_d[ob][:, qc * QC : (qc + 1) * QC], in_=o)
